# revision 9
# baseline (speedup 1.0000x reference)
"""Multi-head dot-product attention (B=2, S=2048, D=2048, H=16, HD=128) with
RoPE + causal mask, sharded over 8 NeuronCores: batch (2) x head-groups (4).

Each core computes 4 heads of one batch element end-to-end (QKV projections,
RoPE, causal softmax attention, output projection); the host sums the four
head-group partials per batch element.

The Q/K/V projections and the output projection run as fp8 (e4m3) DoubleRow
matmuls with a hi+lo residual decomposition: X*W ~ Xhi@Whi + Xhi@Wlo + Xlo@Whi
costs 0.75x the bf16 PE cycles (DoubleRow processes two 128-K tiles per
instruction at 0.5 cycles/row) while carrying ~fp16 accuracy.  Inputs are
pre-scaled (x*16, w*64) on the host so the fp8 residuals stay out of the
subnormal range; the scales are folded into the exp() scale, the row-sum
broadcast constant, and a final host-side divide.

Self-contained: hardcodes all shapes; builds/compiles the Bass program once
per process and runs it via run_bass_kernel_spmd on cores 0-7.
"""

import os
import sys
import types

import ml_dtypes
import numpy as np

B, S, D, H, HD = 2, 2048, 2048, 16, 128
HPC = 4                 # heads per core
HW = HPC * HD           # 512: per-core projection width
NQB = S // 512          # 4 query blocks / token quarters of 512
NKT = S // 128          # 16 key-token tiles of 128
NDC = D // 128          # 16 contraction chunks of 128
NPC = NDC // 2          # 8 chunk-pairs for DoubleRow
N_CORES = 8

SX = 16.0               # host scale on activations before fp8
SW = 64.0               # host scale on wq/wk/wv before fp8
SWO = 64.0              # host scale on wo before fp8
CTX_DIV = 32.0          # ctx divided by this before fp8 (ones_row value);
                        # concentrated softmax rows give ctx ~ v with
                        # |v|~5*1024, so /32 keeps |cxf| < fp8 max 240
# q,k,v come out scaled by SX*SW; exp(scale*st) must see HD^-0.5 * true logits
EXP_SCALE = float(HD) ** -0.5 / (SX * SW) ** 2
# psum out = (ctx_chip/CTX_DIV) @ (wo*SWO) = SX*SW/CTX_DIV*SWO * true
OUT_DIV = SX * SW / CTX_DIV * SWO

BF16 = ml_dtypes.bfloat16
FP8 = ml_dtypes.float8_e4m3

_CACHE = {}


def _install_ntff_hook():
    """The image's antenv lacks axon_hooks, so boot() couldn't register the
    NTFF profile hook; recreate the module + hook so trace=True works."""
    if "antenv.axon_hooks" in sys.modules:
        return
    try:
        import antenv  # noqa: F401
        mod = types.ModuleType("antenv.axon_hooks")
        _h = [None]
        mod.set_axon_ntff_profile_hook = lambda h: _h.__setitem__(0, h)
        mod.get_axon_ntff_profile_hook = lambda: _h[0]
        sys.modules["antenv.axon_hooks"] = mod
        from trn_agent_boot.trn_boot import _ntff_profile_via_ctypes
        mod.set_axon_ntff_profile_hook(
            _ntff_profile_via_ctypes("/opt/axon/libaxon_pjrt.so"))
    except Exception:
        pass


def _build():
    import concourse.mybir as mybir
    import concourse.tile as tile
    from concourse import bacc

    f32 = mybir.dt.float32
    bf16 = mybir.dt.bfloat16
    fp16 = mybir.dt.float16
    fp8 = mybir.dt.float8e4
    Exp = mybir.ActivationFunctionType.Exp
    DR = mybir.MatmulPerfMode.DoubleRow

    nc = bacc.Bacc("TRN2", target_bir_lowering=False, debug=False,
                   enable_asserts=True, num_devices=N_CORES)

    dram = {}
    for name, shape, dt in [
        ("xqT_hi", [D, S], fp8), ("xqT_lo", [D, S], fp8),
        ("xkvT_hi", [D, S], fp8), ("xkvT_lo", [D, S], fp8),
        ("wq_hi", [D, HW], fp8), ("wq_lo", [D, HW], fp8),
        ("wk_hi", [D, HW], fp8), ("wk_lo", [D, HW], fp8),
        ("wv_hi", [D, HW], fp8), ("wv_lo", [D, HW], fp8),
        ("wo_hi", [HW, D], fp8), ("wo_lo", [HW, D], fp8),
        ("sinT", [HD, S], bf16), ("cosT", [HD, S], bf16),
        ("rmatT", [HD, HD], bf16),
        ("ones_col", [128, 1], fp16), ("ones_row", [1, 128], fp16),
        ("maskt", [128, 128], fp16),
    ]:
        dram[name] = nc.dram_tensor(name, shape, dt, kind="ExternalInput").ap()
    outp = nc.dram_tensor("outp", [S, D], bf16, kind="ExternalOutput").ap()

    with tile.TileContext(nc) as tc:
        with (
            tc.tile_pool(name="const", bufs=1) as cpool,
            tc.tile_pool(name="kt", bufs=1) as kt_pool,
            tc.tile_pool(name="qt", bufs=1) as qt_pool,
            tc.tile_pool(name="vsb", bufs=1) as v_pool,
            tc.tile_pool(name="ctxn", bufs=1) as ctx_pool,
            tc.tile_pool(name="wkv", bufs=1) as wkv_pool,
            tc.tile_pool(name="xin", bufs=2) as xpool,
            tc.tile_pool(name="raw", bufs=4) as raw_pool,
            tc.tile_pool(name="t12", bufs=6) as t12_pool,
            tc.tile_pool(name="pp", bufs=8) as ppool,
            tc.tile_pool(name="sacc", bufs=6) as sacc_pool,
            tc.tile_pool(name="rs", bufs=2) as rs_pool,
            tc.tile_pool(name="rcp", bufs=3) as rpool,
            tc.tile_pool(name="cxf", bufs=4) as cxf_pool,
            tc.tile_pool(name="osb", bufs=6) as opool,
            # one PSUM pool for the whole kernel: 4 tags x 2 bufs = 8 banks;
            # no pool-boundary stalls between phases
            tc.tile_pool(name="ps", space="PSUM", bufs=2) as pspool,
        ):
            def load_chunks(pool, name, width, tag, eng, grp=4):
                # [NDC*128, width] fp8 dram -> one [128, NDC*width] sbuf
                # tile, `grp` chunks per DMA (strided AP)
                t = pool.tile([128, NDC * width], fp8, tag=tag,
                              name=name + "_sb")
                dv = dram[name].rearrange("(n p) w -> p n w", p=128)
                for i in range(0, NDC, grp):
                    eng.dma_start(t[:, i * width:(i + grp) * width],
                                  dv[:, i:i + grp, :])
                return t

            def load_pair(pool, base, width, tag_base, eng, grp=4, nch=NDC):
                # interleave hi/lo group DMAs so early chunk-pairs of both
                # arrive first and matmuls can start ASAP
                thi = pool.tile([128, nch * width], fp8, tag=tag_base + "hi",
                                name=base + "_hi_sb")
                tlo = pool.tile([128, nch * width], fp8, tag=tag_base + "lo",
                                name=base + "_lo_sb")
                dhi = dram[base + "_hi"].rearrange("(n p) w -> p n w", p=128)
                dlo = dram[base + "_lo"].rearrange("(n p) w -> p n w", p=128)
                for i in range(0, nch, grp):
                    eng.dma_start(thi[:, i * width:(i + grp) * width],
                                  dhi[:, i:i + grp, :])
                    eng.dma_start(tlo[:, i * width:(i + grp) * width],
                                  dlo[:, i:i + grp, :])
                return thi, tlo

            def load(name, shape, dt=bf16, eng=None):
                t = cpool.tile(shape, dt, tag=name, name=name)
                (eng or nc.gpsimd).dma_start(t[:], dram[name][:])
                return t

            # startup DMA queues: wk on Scalar, sin/cos + wv + wo on GpSimd,
            # rmatT + xkvT q0 on Sync (inside proj_quarter).  rmatT gates the
            # in-order PE stream (first rope rot matmul) so it rides ahead of
            # the x tiles on Sync; sin/cos gate only DVE work.
            rmatT = load("rmatT", [HD, HD], eng=nc.sync)
            wk_hi, wk_lo = load_pair(wkv_pool, "wk", HW, "wk", nc.scalar)
            sinT = load("sinT", [HD, S])
            cosT = load("cosT", [HD, S])
            wv_hi, wv_lo = load_pair(wkv_pool, "wv", HW, "wv", nc.gpsimd)
            maskt = load("maskt", [128, 128], fp16)
            ones_col = load("ones_col", [128, 1], fp16)
            ones_row = load("ones_row", [1, 128], fp16)
            # wo is [HW, D]: 4 chunks of 128 partitions
            wo_hi, wo_lo = load_pair(cpool, "wo", D, "wo", nc.gpsimd,
                                     grp=2, nch=HW // 128)

            # per-head projection outputs (+rope for Q/K)
            kt_sb = [kt_pool.tile([128, S], bf16, tag=f"kt{h}", name=f"kt{h}")
                     for h in range(HPC)]
            qt_sb = [qt_pool.tile([128, S], bf16, tag=f"qt{h}", name=f"qt{h}")
                     for h in range(HPC)]
            v_sb = v_pool.tile([128, NKT * HW], fp16, tag="v", name="v_sb")
            # ctx hi/lo in single tiles so WO can read head-pairs (DoubleRow)
            ctx_hi = ctx_pool.tile([128, HPC * S], fp8, tag="ctxhi",
                                   name="ctx_hi")
            ctx_lo = ctx_pool.tile([128, HPC * S], fp8, tag="ctxlo",
                                   name="ctx_lo")

            def pair3(t, w):
                # [128, NDC*w] tile -> [128, NDC, w] AP for chunk-pair slicing
                return t[:].rearrange("p (n w) -> p n w", w=w)

            def proj_quarter(xbase, tq, whi, wlo, out_tiles, acc_tag, rot_tag,
                             with_v=False):
                """Token-quarter tq of x^T: per-head 512-wide fp8 DoubleRow
                projection (+rope) into out_tiles[h][:, tq*512:...]; for the
                kv pass also the 4 V token-tiles of this quarter."""
                sl = slice(tq * 512, (tq + 1) * 512)
                xhi = xpool.tile([128, NDC * 512], fp8, tag="xhi",
                                 name=f"{xbase}_hi_{tq}")
                xlo = xpool.tile([128, NDC * 512], fp8, tag="xlo",
                                 name=f"{xbase}_lo_{tq}")
                dhi = dram[xbase + "_hi"].rearrange("(n p) s -> p n s", p=128)
                dlo = dram[xbase + "_lo"].rearrange("(n p) s -> p n s", p=128)
                for kc in range(0, NDC, 4):
                    nc.sync.dma_start(
                        xhi[:, kc * 512:(kc + 4) * 512],
                        dhi[:, kc:kc + 4, tq * 512:(tq + 1) * 512])
                    nc.sync.dma_start(
                        xlo[:, kc * 512:(kc + 4) * 512],
                        dlo[:, kc:kc + 4, tq * 512:(tq + 1) * 512])
                xhi3 = pair3(xhi, 512)
                xlo3 = pair3(xlo, 512)
                whi3 = pair3(whi, HW)
                wlo3 = pair3(wlo, HW)
                for h in range(HPC):
                    hsl = slice(h * HD, (h + 1) * HD)
                    ps = pspool.tile([128, 512], f32, tag=acc_tag,
                                     name=f"ps_{xbase}_{tq}_{h}")
                    terms = [(whi3, xhi3), (wlo3, xhi3), (whi3, xlo3)]
                    for ti, (w3, x3) in enumerate(terms):
                        for pc in range(NPC):
                            nc.tensor.matmul(
                                ps[:],
                                lhsT=w3[:, 2 * pc:2 * pc + 2, hsl],
                                rhs=x3[:, 2 * pc:2 * pc + 2, :],
                                start=(ti == 0 and pc == 0),
                                stop=(ti == 2 and pc == NPC - 1),
                                perf_mode=DR)
                    raw = raw_pool.tile([128, 512], bf16, tag="raw",
                                        name=f"raw_{xbase}_{tq}_{h}")
                    nc.scalar.copy(raw[:], ps[:])
                    # rope: out = raw*cos + (R @ raw)*sin
                    rot = pspool.tile([128, 512], f32, tag=rot_tag,
                                      name=f"rot_{xbase}_{tq}_{h}")
                    nc.tensor.matmul(rot[:], lhsT=rmatT[:], rhs=raw[:])
                    t1 = t12_pool.tile([128, 512], bf16, tag="t1", name="t1")
                    nc.vector.tensor_mul(t1[:], rot[:], sinT[:, sl])
                    t2 = t12_pool.tile([128, 512], bf16, tag="t2", name="t2")
                    nc.vector.tensor_mul(t2[:], raw[:], cosT[:, sl])
                    nc.vector.tensor_add(out_tiles[h][:, sl], t1[:], t2[:])
                if with_v:
                    emit_v(tq, xhi3, xlo3)

            def emit_v(tq, xhi3, xlo3):
                wvhi3 = pair3(wv_hi, HW)
                wvlo3 = pair3(wv_lo, HW)
                for ti in range(4):
                    t = tq * 4 + ti
                    tsl = slice(ti * 128, (ti + 1) * 128)
                    ps = pspool.tile([128, 512], f32, tag="C",
                                     name=f"vps_{t}")
                    terms = [(xhi3, wvhi3), (xhi3, wvlo3), (xlo3, wvhi3)]
                    for tt, (x3, w3) in enumerate(terms):
                        for pc in range(NPC):
                            nc.tensor.matmul(
                                ps[:],
                                lhsT=x3[:, 2 * pc:2 * pc + 2, tsl],
                                rhs=w3[:, 2 * pc:2 * pc + 2, :],
                                start=(tt == 0 and pc == 0),
                                stop=(tt == 2 and pc == NPC - 1),
                                perf_mode=DR)
                    nc.scalar.copy(v_sb[:, t * HW:(t + 1) * HW], ps[:])

            def attention_block(qb):
                """Causal attention for query block qb, heads in pairs so
                independent work hides softmax chains; PSUM: ctx=A, st=C,
                s/rb/WO=D (B is left for the concurrent Q-projection)."""
                last = 4 * qb + 3
                for hp in range(2):
                    pair = (2 * hp, 2 * hp + 1)
                    ctx_ps = {h: pspool.tile([128, 512], f32, tag="A",
                                             name=f"ctxps_{h}_{qb}")
                              for h in pair}
                    accs = {h: sacc_pool.tile([128, 512], fp16, tag="acc",
                                              name=f"acc_{h}_{qb}")
                            for h in pair}
                    for kt in range(last + 1):
                        for h in pair:
                            # columns left of the causal diagonal are never
                            # valid: compute S/exp/PV only on [off:512] and
                            # mask just the 128-wide triangular sub-block
                            off = 128 * (kt - 4 * qb) if kt >= 4 * qb else 0
                            st = pspool.tile([128, 512], f32, tag="C",
                                             name=f"st_{h}_{qb}_{kt}")
                            nc.tensor.matmul(
                                st[:, off:],
                                lhsT=kt_sb[h][:, kt * 128:(kt + 1) * 128],
                                rhs=qt_sb[h][:, qb * 512 + off:
                                             (qb + 1) * 512])
                            p = ppool.tile([128, 512], fp16, tag="p",
                                           name=f"p_{h}_{qb}_{kt}")
                            nc.scalar.activation(p[:, off:], st[:, off:],
                                                 Exp, scale=EXP_SCALE)
                            if kt >= 4 * qb:
                                nc.vector.tensor_mul(
                                    p[:, off:off + 128],
                                    p[:, off:off + 128], maskt[:, 0:128])
                            if kt == 0:
                                nc.vector.tensor_copy(accs[h][:], p[:])
                            else:
                                nc.vector.tensor_add(accs[h][:, off:],
                                                     accs[h][:, off:],
                                                     p[:, off:])
                            nc.tensor.matmul(
                                ctx_ps[h][:, off:],
                                lhsT=v_sb[:, kt * HW + h * HD:
                                          kt * HW + (h + 1) * HD],
                                rhs=p[:, off:], start=(kt == 0),
                                stop=(kt == last))
                    for h in pair:
                        s_ps = pspool.tile([1, 512], f32, tag="D",
                                           name=f"sps_{h}_{qb}")
                        nc.tensor.matmul(s_ps[:], lhsT=ones_col[:],
                                         rhs=accs[h][:])
                        s_sb = rs_pool.tile([1, 512], fp16, tag="ssb",
                                            name=f"ssb_{h}_{qb}")
                        nc.scalar.copy(s_sb[:], s_ps[:])
                        # ones_row holds CTX_DIV, so rb = CTX_DIV * rowsum
                        rb_ps = pspool.tile([128, 512], f32, tag="D",
                                            name=f"rbps_{h}_{qb}")
                        nc.tensor.matmul(rb_ps[:], lhsT=ones_row[:],
                                         rhs=s_sb[:])
                        rb_sb = rpool.tile([128, 512], f32, tag="rb",
                                           name=f"rbsb_{h}_{qb}")
                        nc.vector.reciprocal_approx_fast(rb_sb[:], rb_ps[:])
                        # ctx scaled into fp8 range, split hi+lo on Pool
                        cxf = cxf_pool.tile([128, 512], fp16, tag="cxf",
                                            name=f"cxf_{h}_{qb}")
                        nc.vector.tensor_mul(cxf[:], ctx_ps[h][:], rb_sb[:])
                        csl = slice(h * S + qb * 512, h * S + (qb + 1) * 512)
                        nc.gpsimd.tensor_copy(ctx_hi[:, csl], cxf[:])
                        nc.gpsimd.tensor_sub(ctx_lo[:, csl], cxf[:],
                                             ctx_hi[:, csl])

            ctx_hi3 = ctx_hi[:].rearrange("p (h s) -> p h s", s=S)
            ctx_lo3 = ctx_lo[:].rearrange("p (h s) -> p h s", s=S)
            wo_hi3 = wo_hi[:].rearrange("p (h d) -> p h d", d=D)
            wo_lo3 = wo_lo[:].rearrange("p (h d) -> p h d", d=D)

            def wo_block(qb):
                for qt in range(qb * 4, qb * 4 + 4):
                    qsl = slice(qt * 128, (qt + 1) * 128)
                    for db in range(NQB):
                        dsl = slice(db * 512, (db + 1) * 512)
                        ps = pspool.tile([128, 512], f32, tag="D",
                                         name=f"ops_{qt}_{db}")
                        terms = [(ctx_hi3, wo_hi3), (ctx_hi3, wo_lo3),
                                 (ctx_lo3, wo_hi3)]
                        for tt, (c3, w3) in enumerate(terms):
                            for hp in range(2):
                                nc.tensor.matmul(
                                    ps[:],
                                    lhsT=c3[:, 2 * hp:2 * hp + 2, qsl],
                                    rhs=w3[:, 2 * hp:2 * hp + 2, dsl],
                                    start=(tt == 0 and hp == 0),
                                    stop=(tt == 2 and hp == 1),
                                    perf_mode=DR)
                        osb = opool.tile([128, 512], bf16, tag="o",
                                         name=f"osb_{qt}_{db}")
                        nc.vector.tensor_copy(osb[:], ps[:])
                        nc.sync.dma_start(
                            outp[qt * 128:(qt + 1) * 128, dsl], osb[:])

            # ---- phase 1: K^T + V (stream xkvT) ----
            for tq in range(NQB):
                proj_quarter("xkvT", tq, wk_hi, wk_lo, kt_sb, "A", "B",
                             with_v=True)
            # ---- phase 2: Q^T quarters interleaved with attention + WO:
            # the projection's dense matmuls (tag B) fill PE gaps left by the
            # exp-gated attention stream; WO lags one block so WO(q2) fills
            # attention(q3).
            wq_hi, wq_lo = load_pair(wkv_pool, "wq", HW, "wk", nc.scalar)
            for tq in reversed(range(NQB)):
                proj_quarter("xqT", tq, wq_hi, wq_lo, qt_sb, "B", "B")
                attention_block(tq)
                wo_block(tq)

    nc.compile()
    return nc


def _host_constants():
    # sin/cos tables exactly as the flaxformer reference (fp32 math)
    fraction = np.arange(0, HD, 2, dtype=np.float32) / np.float32(HD)
    timescale = (np.float32(10000.0) ** fraction).astype(np.float32)
    sinusoid = np.einsum(
        "i,j->ij", np.arange(S, dtype=np.float32),
        (np.float32(1.0) / timescale)).astype(np.float32)
    sinusoid = np.concatenate([sinusoid, sinusoid], axis=-1)  # [S, HD]
    sinT = np.sin(sinusoid).astype(np.float32).T.copy()
    cosT = np.cos(sinusoid).astype(np.float32).T.copy()

    # rotate_half as a matmul: rot = R @ x, lhsT = R^T
    R = np.zeros((HD, HD), np.float32)
    for i in range(64):
        R[i, i + 64] = -1.0
        R[i + 64, i] = 1.0

    # causal mask variants for the 4 diagonal sub-blocks: allowed iff
    # q - k >= 0 with q = 512*qb + c, k = 128*(4*qb + v) + r
    # causal triangle for the 128-wide diagonal sub-block: allowed iff c >= r
    r = np.arange(128)[:, None]
    c = np.arange(128)[None, :]
    maskt = (c - r >= 0).astype(np.float32)

    return {
        "sinT": sinT.astype(BF16), "cosT": cosT.astype(BF16),
        "rmatT": R.T.copy().astype(BF16),
        "ones_col": np.ones((128, 1), np.float16),
        "ones_row": np.full((1, 128), CTX_DIV, np.float16),
        "maskt": maskt.astype(np.float16),
    }


def _split_fp8(a, scale):
    """hi/lo fp8 residual pair of a*scale (hi+lo ~ a*scale to ~fp16 acc)."""
    s = (np.asarray(a, np.float32) * np.float32(scale))
    hi = s.astype(FP8)
    lo = (s - hi.astype(np.float32)).astype(FP8)
    return hi, lo


def kernel(inputs_q, inputs_kv, wq, wk, wv, wo, mask=None):
    _install_ntff_hook()
    from concourse import bass_utils

    if "nc" not in _CACHE:
        _CACHE["nc"] = _build()
        _CACHE["consts"] = _host_constants()
    nc = _CACHE["nc"]
    consts = _CACHE["consts"]

    wq2 = np.asarray(wq, np.float32).reshape(D, H * HD)
    wk2 = np.asarray(wk, np.float32).reshape(D, H * HD)
    wv2 = np.asarray(wv, np.float32).reshape(D, H * HD)
    wo2 = np.asarray(wo, np.float32).reshape(H * HD, D)

    xq_hl = [_split_fp8(np.asarray(inputs_q[b], np.float32).T, SX)
             for b in range(B)]
    xkv_hl = [_split_fp8(np.asarray(inputs_kv[b], np.float32).T, SX)
              for b in range(B)]

    in_maps = []
    for c in range(N_CORES):
        b, hg = divmod(c, H // HPC)
        hs = slice(hg * HW, (hg + 1) * HW)
        wq_hi, wq_lo = _split_fp8(wq2[:, hs], SW)
        wk_hi, wk_lo = _split_fp8(wk2[:, hs], SW)
        wv_hi, wv_lo = _split_fp8(wv2[:, hs], SW)
        wo_hi, wo_lo = _split_fp8(wo2[hs, :], SWO)
        in_maps.append({
            "xqT_hi": xq_hl[b][0], "xqT_lo": xq_hl[b][1],
            "xkvT_hi": xkv_hl[b][0], "xkvT_lo": xkv_hl[b][1],
            "wq_hi": wq_hi, "wq_lo": wq_lo,
            "wk_hi": wk_hi, "wk_lo": wk_lo,
            "wv_hi": wv_hi, "wv_lo": wv_lo,
            "wo_hi": wo_hi, "wo_lo": wo_lo,
            **consts,
        })

    trace = bool(int(os.environ.get("KERNEL_TRACE", "0")))
    res = bass_utils.run_bass_kernel_spmd(
        nc, in_maps, core_ids=list(range(N_CORES)), trace=trace)
    _CACHE["last_result"] = res

    out = np.zeros((B, S, D), np.float32)
    for c in range(N_CORES):
        out[c // (H // HPC)] += np.asarray(res.results[c]["outp"], np.float32)
    out /= np.float32(OUT_DIV)
    return out


# revision 11
# speedup vs baseline: 1.3625x; 1.3625x over previous
"""Multi-head dot-product attention (B=2, S=2048, D=2048, H=16, HD=128) with
RoPE + causal mask, sharded over 8 NeuronCores: batch (2) x head-groups (4).

Each core computes 4 heads of one batch element end-to-end (QKV projections,
RoPE, causal softmax attention, output projection); the host sums the four
head-group partials per batch element.

Self-contained: hardcodes all shapes; builds/compiles the Bass program once
per process and runs it via run_bass_kernel_spmd on cores 0-7.
"""

import os
import sys
import types

import ml_dtypes
import numpy as np

B, S, D, H, HD = 2, 2048, 2048, 16, 128
HPC = 4                 # heads per core
HW = HPC * HD           # 512: per-core projection width
NQB = S // 512          # 4 query blocks / token quarters of 512
NKT = S // 128          # 16 key-token tiles of 128
NDC = D // 128          # 16 contraction chunks of 128
N_CORES = 8
SCALE = float(HD) ** -0.5

BF16 = ml_dtypes.bfloat16

_CACHE = {}


def _install_ntff_hook():
    """The image's antenv lacks axon_hooks, so boot() couldn't register the
    NTFF profile hook; recreate the module + hook so trace=True works."""
    if "antenv.axon_hooks" in sys.modules:
        return
    try:
        import antenv  # noqa: F401
        mod = types.ModuleType("antenv.axon_hooks")
        _h = [None]
        mod.set_axon_ntff_profile_hook = lambda h: _h.__setitem__(0, h)
        mod.get_axon_ntff_profile_hook = lambda: _h[0]
        sys.modules["antenv.axon_hooks"] = mod
        from trn_agent_boot.trn_boot import _ntff_profile_via_ctypes
        mod.set_axon_ntff_profile_hook(
            _ntff_profile_via_ctypes("/opt/axon/libaxon_pjrt.so"))
    except Exception:
        pass


def _build():
    import concourse.mybir as mybir
    import concourse.tile as tile
    from concourse import bacc

    f32 = mybir.dt.float32
    bf16 = mybir.dt.bfloat16
    fp16 = mybir.dt.float16
    Exp = mybir.ActivationFunctionType.Exp

    nc = bacc.Bacc("TRN2", target_bir_lowering=False, debug=False,
                   enable_asserts=True, num_devices=N_CORES)

    dram = {}
    for name, shape, dt in [
        ("xqT", [D, S], bf16), ("xkvT", [D, S], bf16),
        ("wq", [D, HW], bf16), ("wk", [D, HW], bf16), ("wv", [D, HW], bf16),
        ("wo", [HW, D], bf16),
        ("sinT", [HD, S], bf16), ("cosT", [HD, S], bf16),
        ("rmatT", [HD, HD], bf16),
        ("ones_col", [128, 1], fp16), ("ones_row", [1, 128], fp16),
        ("maskt", [128, 128], fp16),
    ]:
        dram[name] = nc.dram_tensor(name, shape, dt, kind="ExternalInput").ap()
    outp = nc.dram_tensor("outp", [S, D], bf16, kind="ExternalOutput").ap()

    with tile.TileContext(nc) as tc:
        with (
            tc.tile_pool(name="const", bufs=1) as cpool,
            tc.tile_pool(name="kt", bufs=1) as kt_pool,
            tc.tile_pool(name="qt", bufs=1) as qt_pool,
            tc.tile_pool(name="vsb", bufs=1) as v_pool,
            tc.tile_pool(name="ctxn", bufs=1) as ctx_pool,
            tc.tile_pool(name="wkv", bufs=1) as wkv_pool,
            tc.tile_pool(name="xin", bufs=2) as xpool,
            tc.tile_pool(name="raw", bufs=4) as raw_pool,
            tc.tile_pool(name="t12", bufs=6) as t12_pool,
            tc.tile_pool(name="pp", bufs=8) as ppool,
            tc.tile_pool(name="sacc", bufs=6) as sacc_pool,
            tc.tile_pool(name="rs", bufs=2) as rs_pool,
            tc.tile_pool(name="rcp", bufs=3) as rpool,
            tc.tile_pool(name="osb", bufs=6) as opool,
            # one PSUM pool for the whole kernel: 4 tags x 2 bufs = 8 banks;
            # no pool-boundary stalls between phases
            tc.tile_pool(name="ps", space="PSUM", bufs=2) as pspool,
        ):
            def load_chunks(pool, name, nch, width, tag=None, eng=None):
                # [nch*128, width] dram -> one [128, nch*width] sbuf tile,
                # four chunks per DMA (strided AP) to cut issue overhead
                t = pool.tile([128, nch * width], bf16, tag=tag or name,
                              name=name + "_sb")
                dv = dram[name].rearrange("(n p) w -> p n w", p=128)
                step = 4 if nch % 4 == 0 else 2
                for i in range(0, nch, step):
                    e = eng or nc.sync
                    e.dma_start(t[:, i * width:(i + step) * width],
                                dv[:, i:i + step, :])
                return t

            def load(name, shape, dt=bf16, eng=None):
                t = cpool.tile(shape, dt, tag=name, name=name)
                (eng or nc.gpsimd).dma_start(t[:], dram[name][:])
                return t

            # startup DMA queues: wk on Scalar, sin/cos + wv + wo on GpSimd,
            # rmatT + x quarters on Sync.  rmatT gates the in-order PE stream
            # (first rope rot matmul) so it rides ahead of the x tiles.
            rmatT = load("rmatT", [HD, HD], eng=nc.sync)
            wk_sb = load_chunks(wkv_pool, "wk", NDC, HW, eng=nc.scalar)
            sinT = load("sinT", [HD, S])
            cosT = load("cosT", [HD, S])
            wv_sb = load_chunks(wkv_pool, "wv", NDC, HW, eng=nc.gpsimd)
            maskt = load("maskt", [128, 128], fp16)
            ones_col = load("ones_col", [128, 1], fp16)
            ones_row = load("ones_row", [1, 128], fp16)
            wo_sb = load_chunks(cpool, "wo", HW // 128, D, eng=nc.gpsimd)

            # per-head projection outputs (+rope for Q/K)
            kt_sb = [kt_pool.tile([128, S], bf16, tag=f"kt{h}", name=f"kt{h}")
                     for h in range(HPC)]
            qt_sb = [qt_pool.tile([128, S], bf16, tag=f"qt{h}", name=f"qt{h}")
                     for h in range(HPC)]
            v_sb = v_pool.tile([128, NKT * HW], fp16, tag="v", name="v_sb")
            ctx_sb = [ctx_pool.tile([128, S], bf16, tag=f"ctx{h}",
                                    name=f"ctx{h}") for h in range(HPC)]

            def proj_quarter(xname, tq, w_sb, out_tiles, acc_tag, rot_tag,
                             with_v=False, cold=False):
                """Token-quarter tq of x^T: per-head 512-wide projection
                (+rope) into out_tiles[h][:, tq*512:...]; for the kv pass
                also the 4 V token-tiles of this quarter.  cold=True runs
                chunk-major across all 4 heads so the PE starts as soon as
                the first chunk-group DMA lands (startup only)."""
                sl = slice(tq * 512, (tq + 1) * 512)
                xt = xpool.tile([128, NDC * 512], bf16, tag="xin",
                                name=f"{xname}_{tq}")
                xv = dram[xname].rearrange("(n p) s -> p n s", p=128)
                for kc in range(0, NDC, 4):
                    nc.sync.dma_start(
                        xt[:, kc * 512:(kc + 4) * 512],
                        xv[:, kc:kc + 4, tq * 512:(tq + 1) * 512])

                def mm(ps, h, kc):
                    nc.tensor.matmul(
                        ps[:],
                        lhsT=w_sb[:, kc * HW + h * HD:kc * HW + (h + 1) * HD],
                        rhs=xt[:, kc * 512:(kc + 1) * 512],
                        start=(kc == 0), stop=(kc == NDC - 1))

                raws = {}
                if cold:
                    # psum tags A,A,B,B; rope rot goes to tag C
                    pss = {h: pspool.tile([128, 512], f32,
                                          tag=("A" if h < 2 else "B"),
                                          name=f"ps_{xname}_{tq}_{h}")
                           for h in range(HPC)}
                    for g in range(0, NDC, 4):
                        for h in range(HPC):
                            for kc in range(g, g + 4):
                                mm(pss[h], h, kc)
                    for h in range(HPC):
                        raw = raw_pool.tile([128, 512], bf16, tag="raw",
                                            name=f"raw_{xname}_{tq}_{h}")
                        nc.scalar.copy(raw[:], pss[h][:])
                        raws[h] = raw
                else:
                    for h in range(HPC):
                        ps = pspool.tile([128, 512], f32, tag=acc_tag,
                                         name=f"ps_{xname}_{tq}_{h}")
                        for kc in range(NDC):
                            mm(ps, h, kc)
                        raw = raw_pool.tile([128, 512], bf16, tag="raw",
                                            name=f"raw_{xname}_{tq}_{h}")
                        nc.scalar.copy(raw[:], ps[:])
                        raws[h] = raw
                for h in range(HPC):
                    raw = raws[h]
                    # rope: out = raw*cos + (R @ raw)*sin
                    rot = pspool.tile([128, 512], f32, tag=rot_tag,
                                      name=f"rot_{xname}_{tq}_{h}")
                    nc.tensor.matmul(rot[:], lhsT=rmatT[:], rhs=raw[:])
                    t1 = t12_pool.tile([128, 512], bf16, tag="t1", name="t1")
                    nc.vector.tensor_mul(t1[:], rot[:], sinT[:, sl])
                    t2 = t12_pool.tile([128, 512], bf16, tag="t2", name="t2")
                    nc.vector.tensor_mul(t2[:], raw[:], cosT[:, sl])
                    nc.vector.tensor_add(out_tiles[h][:, sl], t1[:], t2[:])
                if with_v:
                    emit_v(tq, xt)
                return xt

            def emit_v(tq, xt):
                for ti in range(4):
                    t = tq * 4 + ti
                    ps = pspool.tile([128, 512], f32, tag="C",
                                     name=f"vps_{t}")
                    for kc in range(NDC):
                        nc.tensor.matmul(
                            ps[:],
                            lhsT=xt[:, kc * 512 + ti * 128:
                                    kc * 512 + (ti + 1) * 128],
                            rhs=wv_sb[:, kc * HW:(kc + 1) * HW],
                            start=(kc == 0), stop=(kc == NDC - 1))
                    nc.scalar.copy(v_sb[:, t * HW:(t + 1) * HW], ps[:])

            def attention_block(qb):
                """Causal attention for query block qb, heads in pairs so
                independent work hides softmax chains; PSUM: ctx=A, st=C,
                s/rb/WO=D (B is left for the concurrent Q-projection)."""
                qsl = slice(qb * 512, (qb + 1) * 512)
                last = 4 * qb + 3
                for hp in range(2):
                    pair = (2 * hp, 2 * hp + 1)
                    ctx_ps = {h: pspool.tile([128, 512], f32, tag="A",
                                             name=f"ctxps_{h}_{qb}")
                              for h in pair}
                    accs = {h: sacc_pool.tile([128, 512], fp16, tag="acc",
                                              name=f"acc_{h}_{qb}")
                            for h in pair}
                    for kt in range(last + 1):
                        for h in pair:
                            # columns left of the causal diagonal are never
                            # valid: compute S/exp/PV only on [off:512] and
                            # mask just the 128-wide triangular sub-block
                            off = 128 * (kt - 4 * qb) if kt >= 4 * qb else 0
                            st = pspool.tile([128, 512], f32, tag="C",
                                             name=f"st_{h}_{qb}_{kt}")
                            nc.tensor.matmul(
                                st[:, off:],
                                lhsT=kt_sb[h][:, kt * 128:(kt + 1) * 128],
                                rhs=qt_sb[h][:, qb * 512 + off:
                                             (qb + 1) * 512])
                            p = ppool.tile([128, 512], fp16, tag="p",
                                           name=f"p_{h}_{qb}_{kt}")
                            nc.scalar.activation(p[:, off:], st[:, off:],
                                                 Exp, scale=SCALE)
                            if kt >= 4 * qb:
                                nc.vector.tensor_mul(
                                    p[:, off:off + 128],
                                    p[:, off:off + 128], maskt[:])
                            if kt == 0:
                                nc.vector.tensor_copy(accs[h][:], p[:])
                            else:
                                nc.vector.tensor_add(accs[h][:, off:],
                                                     accs[h][:, off:],
                                                     p[:, off:])
                            nc.tensor.matmul(
                                ctx_ps[h][:, off:],
                                lhsT=v_sb[:, kt * HW + h * HD:
                                          kt * HW + (h + 1) * HD],
                                rhs=p[:, off:], start=(kt == 0),
                                stop=(kt == last))
                    for h in pair:
                        s_ps = pspool.tile([1, 512], f32, tag="D",
                                           name=f"sps_{h}_{qb}")
                        nc.tensor.matmul(s_ps[:], lhsT=ones_col[:],
                                         rhs=accs[h][:])
                        s_sb = rs_pool.tile([1, 512], fp16, tag="ssb",
                                            name=f"ssb_{h}_{qb}")
                        nc.scalar.copy(s_sb[:], s_ps[:])
                        rb_ps = pspool.tile([128, 512], f32, tag="D",
                                            name=f"rbps_{h}_{qb}")
                        nc.tensor.matmul(rb_ps[:], lhsT=ones_row[:],
                                         rhs=s_sb[:])
                        rb_sb = rpool.tile([128, 512], f32, tag="rb",
                                           name=f"rbsb_{h}_{qb}")
                        nc.vector.reciprocal_approx_fast(rb_sb[:], rb_ps[:])
                        nc.vector.tensor_mul(ctx_sb[h][:, qsl],
                                             ctx_ps[h][:], rb_sb[:])

            def wo_block(qb):
                for qt in range(qb * 4, qb * 4 + 4):
                    for db in range(NQB):
                        ps = pspool.tile([128, 512], f32, tag="D",
                                         name=f"ops_{qt}_{db}")
                        for h in range(HPC):
                            nc.tensor.matmul(
                                ps[:],
                                lhsT=ctx_sb[h][:, qt * 128:(qt + 1) * 128],
                                rhs=wo_sb[:, h * D + db * 512:
                                          h * D + (db + 1) * 512],
                                start=(h == 0), stop=(h == HPC - 1))
                        osb = opool.tile([128, 512], bf16, tag="o",
                                         name=f"osb_{qt}_{db}")
                        nc.vector.tensor_copy(osb[:], ps[:])
                        nc.sync.dma_start(
                            outp[qt * 128:(qt + 1) * 128,
                                 db * 512:(db + 1) * 512], osb[:])

            # ---- phase 1: K^T + V (stream xkvT) ----
            for tq in range(NQB):
                proj_quarter("xkvT", tq, wk_sb, kt_sb, "A", "B", with_v=True,
                             cold=(tq == 0))
            # ---- phase 2: Q^T quarters interleaved with attention + WO:
            # the projection's dense matmuls (tag B) fill PE gaps left by the
            # exp-gated attention stream; WO lags one block so WO(q2) fills
            # attention(q3).
            wq_sb = load_chunks(wkv_pool, "wq", NDC, HW, tag="wk",
                                eng=nc.scalar)
            for tq in reversed(range(NQB)):
                proj_quarter("xqT", tq, wq_sb, qt_sb, "B", "B")
                attention_block(tq)
                wo_block(tq)

    nc.compile()
    return nc


def _host_constants():
    # sin/cos tables exactly as the flaxformer reference (fp32 math)
    fraction = np.arange(0, HD, 2, dtype=np.float32) / np.float32(HD)
    timescale = (np.float32(10000.0) ** fraction).astype(np.float32)
    sinusoid = np.einsum(
        "i,j->ij", np.arange(S, dtype=np.float32),
        (np.float32(1.0) / timescale)).astype(np.float32)
    sinusoid = np.concatenate([sinusoid, sinusoid], axis=-1)  # [S, HD]
    sinT = np.sin(sinusoid).astype(np.float32).T.copy()
    cosT = np.cos(sinusoid).astype(np.float32).T.copy()

    # rotate_half as a matmul: rot = R @ x, lhsT = R^T
    R = np.zeros((HD, HD), np.float32)
    for i in range(64):
        R[i, i + 64] = -1.0
        R[i + 64, i] = 1.0

    # causal triangle for the 128-wide diagonal sub-block: allowed iff c >= r
    r = np.arange(128)[:, None]
    c = np.arange(128)[None, :]
    maskt = (c - r >= 0).astype(np.float32)

    return {
        "sinT": sinT.astype(BF16), "cosT": cosT.astype(BF16),
        "rmatT": R.T.copy().astype(BF16),
        "ones_col": np.ones((128, 1), np.float16),
        "ones_row": np.ones((1, 128), np.float16),
        "maskt": maskt.astype(np.float16),
    }


def kernel(inputs_q, inputs_kv, wq, wk, wv, wo, mask=None):
    _install_ntff_hook()
    from concourse import bass_utils

    if "nc" not in _CACHE:
        _CACHE["nc"] = _build()
        _CACHE["consts"] = _host_constants()
    nc = _CACHE["nc"]
    consts = _CACHE["consts"]

    wq2 = np.asarray(wq, np.float32).reshape(D, H * HD)
    wk2 = np.asarray(wk, np.float32).reshape(D, H * HD)
    wv2 = np.asarray(wv, np.float32).reshape(D, H * HD)
    wo2 = np.asarray(wo, np.float32).reshape(H * HD, D)
    xq = np.asarray(inputs_q, np.float32)
    xkv = np.asarray(inputs_kv, np.float32)

    in_maps = []
    for c in range(N_CORES):
        b, hg = divmod(c, H // HPC)
        hs = slice(hg * HW, (hg + 1) * HW)
        in_maps.append({
            "xqT": np.ascontiguousarray(xq[b].T).astype(BF16),
            "xkvT": np.ascontiguousarray(xkv[b].T).astype(BF16),
            "wq": wq2[:, hs].astype(BF16),
            "wk": wk2[:, hs].astype(BF16),
            "wv": wv2[:, hs].astype(BF16),
            "wo": wo2[hs, :].astype(BF16),
            **consts,
        })

    trace = bool(int(os.environ.get("KERNEL_TRACE", "0")))
    res = bass_utils.run_bass_kernel_spmd(
        nc, in_maps, core_ids=list(range(N_CORES)), trace=trace)
    _CACHE["last_result"] = res

    out = np.zeros((B, S, D), np.float32)
    for c in range(N_CORES):
        out[c // (H // HPC)] += np.asarray(res.results[c]["outp"], np.float32)
    return out
